# revision 16
# baseline (speedup 1.0000x reference)
"""GQA attention kernel for 8 TRN2 NeuronCores.

Sharding: data-parallel over batch (B=2) x tensor-parallel over heads (4-way).
Core i handles batch i//4 and head-shard i%4 (8 query heads = 2 KV groups).
Out-proj is row-sharded; the 4 partial [S,D] outputs per batch are summed on
the host (cheap unshard step), bo added once.

Device kernel (per core, all bf16 matmuls, f32 PSUM):
  QT = Wq_sh.T @ xT          [512, S]   (x pre-transposed on host)
  KT = Wk_sh.T @ kvT         [128, S]
  V  = kvT.T  @ Wv_sh        [S, 128] -> per-group V_aug [S, 64+1] (ones col)
  per (head, q-chunk 512): scores^T chunks [128 keys, 512 q] -> exp (no max
  subtraction; scores are O(1)) -> causal mask via sliding window of a
  precomputed [128,1024] 0/1 mask -> PV accumulate with ones-row giving
  softmax sums in row 64 -> normalize via reciprocal + ones-outer-product
  broadcast -> OT [512, S] -> out_partial = OT.T @ Wo_sh  [S, D] f32.
"""

import numpy as np

B, S, D = 2, 2048, 2048
H, G, HD, GS = 32, 8, 64, 4
HPC = 8     # query heads per core
GPC = 2     # kv groups per core
NCORES = 8
SCALE = 0.125  # 1/sqrt(64)

_CACHE = {}


def _build():
    import concourse.bass as bass
    import concourse.tile as tile
    from concourse import bacc, mybir

    f32 = mybir.dt.float32
    bf16 = mybir.dt.bfloat16
    AF = mybir.ActivationFunctionType
    ALU = mybir.AluOpType

    nc = bacc.Bacc("TRN2", target_bir_lowering=False, debug=False,
                   num_devices=NCORES)

    xT_d = nc.declare_dram_parameter("xT", [D, S], bf16, isOutput=False)
    kvT_d = nc.declare_dram_parameter("kvT", [D, S], bf16, isOutput=False)
    wq_d = nc.declare_dram_parameter("wq", [D, 512], bf16, isOutput=False)
    wk_d = nc.declare_dram_parameter("wk", [D, 128], bf16, isOutput=False)
    wv_d = nc.declare_dram_parameter("wv", [D, 128], bf16, isOutput=False)
    wo_d = nc.declare_dram_parameter("wo", [512, D], bf16, isOutput=False)
    bq_d = nc.declare_dram_parameter("bq", [128, 4], f32, isOutput=False)
    bk_d = nc.declare_dram_parameter("bk", [128, 1], f32, isOutput=False)
    bvt_d = nc.declare_dram_parameter("bvt", [128, 2 * 64], f32, isOutput=False)
    m0_d = nc.declare_dram_parameter("m0", [128, 1024], bf16, isOutput=False)
    out_d = nc.declare_dram_parameter("out", [S, D], f32, isOutput=True)

    with tile.TileContext(nc) as tc:
        with (
            tc.tile_pool(name="persist", bufs=1) as persist,
            tc.tile_pool(name="stream", bufs=2) as stream,
            tc.tile_pool(name="small", bufs=2) as small,
            tc.tile_pool(name="probs", bufs=4) as probs_pool,
            tc.tile_pool(name="ps_rot", bufs=3, space="PSUM") as ps_rot,
            tc.tile_pool(name="ps_o", bufs=2, space="PSUM") as ps_o,
            tc.tile_pool(name="ps_b", bufs=2, space="PSUM") as ps_b,
        ):
            # ---- resident weights (one wide tile per weight, sliced) ----
            wq_sb = persist.tile([128, 16 * 512], bf16, tag="wq")   # chunk c at c*512
            wk_sb = persist.tile([128, 16 * 128], bf16, tag="wk")
            wv_sb = persist.tile([128, 16 * 128], bf16, tag="wv")
            wo_sb = persist.tile([128, 4 * 2048], bf16, tag="wo")
            for c in range(16):
                nc.sync.dma_start(out=wq_sb[:, c * 512:(c + 1) * 512],
                                  in_=wq_d[c * 128:(c + 1) * 128, :])
                nc.sync.dma_start(out=wk_sb[:, c * 128:(c + 1) * 128],
                                  in_=wk_d[c * 128:(c + 1) * 128, :])
                nc.sync.dma_start(out=wv_sb[:, c * 128:(c + 1) * 128],
                                  in_=wv_d[c * 128:(c + 1) * 128, :])
            for c in range(4):
                nc.sync.dma_start(out=wo_sb[:, c * 2048:(c + 1) * 2048],
                                  in_=wo_d[c * 128:(c + 1) * 128, :])
            m0_sb = persist.tile([128, 1024], bf16, tag="m0")
            nc.sync.dma_start(out=m0_sb[:], in_=m0_d[:, :])
            bq_sb = persist.tile([128, 4], f32, tag="bq")
            nc.sync.dma_start(out=bq_sb[:], in_=bq_d[:, :])
            bk_sb = persist.tile([128, 1], f32, tag="bk")
            nc.sync.dma_start(out=bk_sb[:], in_=bk_d[:, :])
            bvt_sb = persist.tile([128, 2 * 64], f32, tag="bvt")
            nc.sync.dma_start(out=bvt_sb[:], in_=bvt_d[:, :])
            ones_sb = persist.tile([1, 64], bf16, tag="ones")
            nc.vector.memset(ones_sb[:], 1.0)

            # ---- resident projection outputs ----
            qt_sb = persist.tile([128, 4 * 2048], bf16, tag="qt")   # chunk hc at hc*2048
            kt_sb = persist.tile([128, S], bf16, tag="kt")
            vaug_sb = persist.tile([128, 2 * 16 * 65], bf16, tag="vaug")  # [gl*1040 + tok*65]
            ot_sb = persist.tile([128, 4 * 2048], bf16, tag="ot")

            # ---- K/V projections (kv token chunks of 512) ----
            for tch in range(4):
                kvt = stream.tile([128, 16 * 512], bf16, tag="xs", name="kvt")
                for c in range(16):
                    nc.sync.dma_start(
                        out=kvt[:, c * 512:(c + 1) * 512],
                        in_=kvT_d[c * 128:(c + 1) * 128, tch * 512:(tch + 1) * 512])
                kps = ps_rot.tile([128, 512], f32, tag="rot", name="kps")
                for c in range(16):
                    nc.tensor.matmul(kps[:],
                                     lhsT=wk_sb[:, c * 128:(c + 1) * 128],
                                     rhs=kvt[:, c * 512:(c + 1) * 512],
                                     start=(c == 0), stop=(c == 15))
                nc.vector.tensor_scalar(
                    kt_sb[:, tch * 512:(tch + 1) * 512], kps[:],
                    bk_sb[:, 0:1], None, ALU.add)
                for tt in range(4):
                    tok = tch * 4 + tt
                    vps = ps_rot.tile([128, 128], f32, tag="rot", name="vps")
                    for c in range(16):
                        nc.tensor.matmul(
                            vps[:],
                            lhsT=kvt[:, c * 512 + tt * 128:c * 512 + (tt + 1) * 128],
                            rhs=wv_sb[:, c * 128:(c + 1) * 128],
                            start=(c == 0), stop=(c == 15))
                    for gl in range(2):
                        base = gl * 1040 + tok * 65
                        nc.vector.tensor_tensor(
                            vaug_sb[:, base:base + 64],
                            vps[:, gl * 64:(gl + 1) * 64],
                            bvt_sb[:, gl * 64:(gl + 1) * 64], ALU.add)
                        nc.vector.memset(vaug_sb[:, base + 64:base + 65], 1.0)

            # ---- Q projection (q chunks of 512) ----
            for qch in range(4):
                xt = stream.tile([128, 16 * 512], bf16, tag="xs", name="xt")
                for c in range(16):
                    nc.sync.dma_start(
                        out=xt[:, c * 512:(c + 1) * 512],
                        in_=xT_d[c * 128:(c + 1) * 128, qch * 512:(qch + 1) * 512])
                for hc in range(4):
                    qps = ps_rot.tile([128, 512], f32, tag="rot", name="qps")
                    for c in range(16):
                        nc.tensor.matmul(
                            qps[:],
                            lhsT=wq_sb[:, c * 512 + hc * 128:c * 512 + (hc + 1) * 128],
                            rhs=xt[:, c * 512:(c + 1) * 512],
                            start=(c == 0), stop=(c == 15))
                    nc.vector.tensor_scalar(
                        qt_sb[:, hc * 2048 + qch * 512:hc * 2048 + (qch + 1) * 512],
                        qps[:], bq_sb[:, hc:hc + 1], None, ALU.add)

            # ---- attention ----
            # head h: chunk hc=h%4 at row-half hr=(h//4)*64 -- host permutes
            # Wq cols / Wo rows so Q rows align with K group rows (matmul
            # requires lhsT.base_partition == rhs.base_partition).
            for h in range(HPC):
                gl = h // 4
                hc, hr = h % 4, gl * 64
                for jq in range(4):
                    ops = ps_o.tile([65, 512], f32, tag="ops", name="ops")
                    nkc = 4 * jq + 4
                    for kci in range(nkc):
                        kc = kci * 128
                        sps = ps_rot.tile([128, 512], f32, tag="rot", name="sps")
                        nc.tensor.matmul(
                            sps[:],
                            lhsT=kt_sb[gl * 64:(gl + 1) * 64, kc:kc + 128],
                            rhs=qt_sb[hr:hr + 64,
                                      hc * 2048 + jq * 512:hc * 2048 + (jq + 1) * 512],
                            start=True, stop=True)
                        pt = probs_pool.tile([128, 512], bf16, tag="pt", name="pt")
                        nc.scalar.activation(pt[:], sps[:], AF.Exp, scale=SCALE)
                        if kci >= 4 * jq:
                            m = kc - jq * 512  # 0,128,256,384
                            nc.vector.tensor_tensor(
                                pt[:], pt[:], m0_sb[:, 512 - m:1024 - m],
                                ALU.mult)
                        vbase = gl * 1040 + kci * 65
                        nc.tensor.matmul(
                            ops[:], lhsT=vaug_sb[:, vbase:vbase + 65],
                            rhs=pt[:], start=(kci == 0), stop=(kci == nkc - 1))
                    rs = small.tile([1, 512], f32, tag="rs")
                    nc.vector.reciprocal(rs[:], ops[64:65, :])
                    rsb = small.tile([1, 512], bf16, tag="rsb")
                    nc.vector.tensor_copy(rsb[:], rs[:])
                    bps = ps_b.tile([64, 512], f32, tag="bps", name="bps")
                    nc.tensor.matmul(bps[:], lhsT=ones_sb[:], rhs=rsb[:],
                                     start=True, stop=True)
                    bsb = small.tile([64, 512], f32, tag="bsb", name="bsb")
                    nc.vector.tensor_copy(bsb[:], bps[:])
                    nc.vector.tensor_tensor(
                        ot_sb[hr:hr + 64,
                              hc * 2048 + jq * 512:hc * 2048 + (jq + 1) * 512],
                        ops[0:64, :], bsb[:], ALU.mult)

            # ---- out projection ----
            for qt_i in range(16):
                qs = slice(qt_i * 128, (qt_i + 1) * 128)
                for cc in range(4):
                    outp = ps_rot.tile([128, 512], f32, tag="rot", name="outp")
                    for c in range(4):
                        nc.tensor.matmul(
                            outp[:],
                            lhsT=ot_sb[:, c * 2048 + qt_i * 128:c * 2048 + (qt_i + 1) * 128],
                            rhs=wo_sb[:, c * 2048 + cc * 512:c * 2048 + (cc + 1) * 512],
                            start=(c == 0), stop=(c == 3))
                    osb = stream.tile([128, 512], f32, tag="osb", name="osb")
                    nc.vector.tensor_copy(osb[:], outp[:])
                    nc.sync.dma_start(
                        out=out_d[qs, cc * 512:(cc + 1) * 512], in_=osb[:])
    nc.finalize()
    return nc


def _get_nc():
    if "nc" not in _CACHE:
        _CACHE["nc"] = _build()
    return _CACHE["nc"]


def kernel(**inputs):
    out, _ = _run(inputs, trace=False)
    return out


def _run(inputs, trace=False):
    import ml_dtypes
    from concourse.bass_utils import run_bass_kernel_spmd

    x = np.asarray(inputs["x"], np.float32)
    kv = np.asarray(inputs["kv"], np.float32)
    Wq = np.asarray(inputs["Wq"], np.float32)
    bq = np.asarray(inputs["bq"], np.float32)
    Wk = np.asarray(inputs["Wk"], np.float32)
    bk = np.asarray(inputs["bk"], np.float32)
    Wv = np.asarray(inputs["Wv"], np.float32)
    bv = np.asarray(inputs["bv"], np.float32)
    Wo = np.asarray(inputs["Wo"], np.float32)
    bo = np.asarray(inputs["bo"], np.float32)

    bf = ml_dtypes.bfloat16
    M0 = (np.arange(1024)[None, :] >= (np.arange(128)[:, None] + 512)
          ).astype(bf)

    # head-dim permutation: chunk c = [local head c | local head 4+c]
    # so each head's Q rows sit at the partition half of its KV group.
    hperm = np.concatenate(
        [np.concatenate([np.arange(c * 64, c * 64 + 64),
                         np.arange((4 + c) * 64, (4 + c) * 64 + 64)])
         for c in range(4)])  # [512] permutation of local head dims

    in_maps = []
    for core in range(NCORES):
        b, t = core // 4, core % 4
        bv_sh = bv[t * 128:(t + 1) * 128]
        bvt = np.broadcast_to(bv_sh[None, :], (128, 128)).astype(np.float32)
        wq_sh = Wq[:, t * 512:(t + 1) * 512][:, hperm]
        wo_sh = Wo[t * 512:(t + 1) * 512, :][hperm, :]
        bq_sh = bq[t * 512:(t + 1) * 512][hperm]
        in_maps.append({
            "xT": np.ascontiguousarray(x[b].T).astype(bf),
            "kvT": np.ascontiguousarray(kv[b].T).astype(bf),
            "wq": wq_sh.astype(bf),
            "wk": Wk[:, t * 128:(t + 1) * 128].astype(bf),
            "wv": Wv[:, t * 128:(t + 1) * 128].astype(bf),
            "wo": np.ascontiguousarray(wo_sh).astype(bf),
            "bq": np.ascontiguousarray(bq_sh.reshape(4, 128).T),
            "bk": bk[t * 128:(t + 1) * 128].reshape(128, 1).copy(),
            "bvt": np.ascontiguousarray(bvt),
            "m0": M0,
        })

    nc = _get_nc()
    res = run_bass_kernel_spmd(nc, in_maps, core_ids=list(range(NCORES)),
                               trace=trace)
    parts = [np.asarray(res.results[i]["out"], np.float32)
             for i in range(NCORES)]
    out = np.stack([parts[0] + parts[1] + parts[2] + parts[3],
                    parts[4] + parts[5] + parts[6] + parts[7]])
    out += bo[None, None, :]
    return out.astype(np.float32), res


# revision 18
# speedup vs baseline: 1.6683x; 1.6683x over previous
"""GQA attention kernel for 8 TRN2 NeuronCores.

Sharding: data-parallel over batch (B=2) x tensor-parallel over heads (4-way).
Core i handles batch i//4 and head-shard i%4 (8 query heads = 2 KV groups).
Out-proj is row-sharded; the 4 partial [S,D] outputs per batch are summed on
the host (cheap unshard step), bo added once.

Device kernel (per core, all bf16 matmuls, f32 PSUM):
  QT = Wq_sh.T @ xT          [512, S]   (x pre-transposed on host)
  KT = Wk_sh.T @ kvT         [128, S]
  V  = kvT.T  @ Wv_sh        [S, 128] -> per-group V_aug [S, 64+1] (ones col)
  per (head, q-chunk 512): scores^T chunks [128 keys, 512 q] -> exp (no max
  subtraction; scores are O(1)) -> causal mask via sliding window of a
  precomputed [128,1024] 0/1 mask -> PV accumulate with ones-row giving
  softmax sums in row 64 -> normalize via reciprocal + ones-outer-product
  broadcast -> OT [512, S] -> out_partial = OT.T @ Wo_sh  [S, D] f32.
"""

import numpy as np

B, S, D = 2, 2048, 2048
H, G, HD, GS = 32, 8, 64, 4
HPC = 8     # query heads per core
GPC = 2     # kv groups per core
NCORES = 8
SCALE = 0.125  # 1/sqrt(64)

_CACHE = {}


def _build():
    import concourse.bass as bass
    import concourse.tile as tile
    from concourse import bacc, mybir

    f32 = mybir.dt.float32
    bf16 = mybir.dt.bfloat16
    AF = mybir.ActivationFunctionType
    ALU = mybir.AluOpType

    nc = bacc.Bacc("TRN2", target_bir_lowering=False, debug=False,
                   num_devices=NCORES)

    xT_d = nc.declare_dram_parameter("xT", [D, S], bf16, isOutput=False)
    kvT_d = nc.declare_dram_parameter("kvT", [D, S], bf16, isOutput=False)
    wq_d = nc.declare_dram_parameter("wq", [D, 512], bf16, isOutput=False)
    wk_d = nc.declare_dram_parameter("wk", [D, 128], bf16, isOutput=False)
    wv_d = nc.declare_dram_parameter("wv", [D, 128], bf16, isOutput=False)
    wo_d = nc.declare_dram_parameter("wo", [512, D], bf16, isOutput=False)
    bq_d = nc.declare_dram_parameter("bq", [128, 4], f32, isOutput=False)
    bk_d = nc.declare_dram_parameter("bk", [128, 1], f32, isOutput=False)
    bvt_d = nc.declare_dram_parameter("bvt", [128, 2 * 64], f32, isOutput=False)
    m0_d = nc.declare_dram_parameter("m0", [128, 1024], bf16, isOutput=False)
    out_d = nc.declare_dram_parameter("out", [S, D], f32, isOutput=True)

    with tile.TileContext(nc) as tc:
        with (
            tc.tile_pool(name="persist", bufs=1) as persist,
            tc.tile_pool(name="stream", bufs=3) as stream,
            tc.tile_pool(name="small", bufs=3) as small,
            tc.tile_pool(name="probs", bufs=6) as probs_pool,
            tc.tile_pool(name="ps_s", bufs=3, space="PSUM") as ps_s,
            tc.tile_pool(name="ps_proj", bufs=2, space="PSUM") as ps_proj,
            tc.tile_pool(name="ps_o", bufs=2, space="PSUM") as ps_o,
            tc.tile_pool(name="ps_b", bufs=1, space="PSUM") as ps_b,
        ):
            # ---- resident weights (one wide tile per weight, sliced) ----
            wq_sb = persist.tile([128, 16 * 512], bf16, tag="wq")   # chunk c at c*512
            wk_sb = persist.tile([128, 16 * 128], bf16, tag="wk")
            wv_sb = persist.tile([128, 16 * 128], bf16, tag="wv")
            wo_sb = persist.tile([128, 4 * 2048], bf16, tag="wo")
            for c in range(16):
                nc.sync.dma_start(out=wq_sb[:, c * 512:(c + 1) * 512],
                                  in_=wq_d[c * 128:(c + 1) * 128, :])
                nc.sync.dma_start(out=wk_sb[:, c * 128:(c + 1) * 128],
                                  in_=wk_d[c * 128:(c + 1) * 128, :])
                nc.sync.dma_start(out=wv_sb[:, c * 128:(c + 1) * 128],
                                  in_=wv_d[c * 128:(c + 1) * 128, :])
            for c in range(4):
                nc.sync.dma_start(out=wo_sb[:, c * 2048:(c + 1) * 2048],
                                  in_=wo_d[c * 128:(c + 1) * 128, :])
            m0_sb = persist.tile([128, 1024], bf16, tag="m0")
            nc.sync.dma_start(out=m0_sb[:], in_=m0_d[:, :])
            bq_sb = persist.tile([128, 4], f32, tag="bq")
            nc.sync.dma_start(out=bq_sb[:], in_=bq_d[:, :])
            bk_sb = persist.tile([128, 1], f32, tag="bk")
            nc.sync.dma_start(out=bk_sb[:], in_=bk_d[:, :])
            bvt_sb = persist.tile([128, 2 * 64], f32, tag="bvt")
            nc.sync.dma_start(out=bvt_sb[:], in_=bvt_d[:, :])
            ones_sb = persist.tile([1, 64], bf16, tag="ones")
            nc.vector.memset(ones_sb[:], 1.0)

            # ---- resident projection outputs ----
            qt_sb = persist.tile([128, 4 * 2048], bf16, tag="qt")   # chunk hc at hc*2048
            kt_sb = persist.tile([128, S], bf16, tag="kt")
            vaug_sb = persist.tile([128, 2 * 16 * 65], bf16, tag="vaug")  # [gl*1040+tok*65]
            ot_sb = persist.tile([128, 4 * 2048], bf16, tag="ot")

            # ---- chain emitters (as thunk lists for PE-filler interleave) ----
            def kv_chain_thunks(tch):
                """K/V projection for kv token chunk tch: DMA + KT + V."""
                th = []
                state = {}

                def dma():
                    kvt = stream.tile([128, 16 * 512], bf16, tag="xs", name="kvt")
                    for c in range(16):
                        nc.sync.dma_start(
                            out=kvt[:, c * 512:(c + 1) * 512],
                            in_=kvT_d[c * 128:(c + 1) * 128,
                                      tch * 512:(tch + 1) * 512])
                    state["kvt"] = kvt
                    state["kps"] = ps_proj.tile([128, 512], f32, tag="proj",
                                                name="kps")
                th.append(dma)

                def kmm(c):
                    nc.tensor.matmul(
                        state["kps"][:], lhsT=wk_sb[:, c * 128:(c + 1) * 128],
                        rhs=state["kvt"][:, c * 512:(c + 1) * 512],
                        start=(c == 0), stop=(c == 15))
                    if c == 15:
                        nc.vector.tensor_scalar(
                            kt_sb[:, tch * 512:(tch + 1) * 512], state["kps"][:],
                            bk_sb[:, 0:1], None, ALU.add)
                for c in range(16):
                    th.append(lambda c=c: kmm(c))

                def vmm(tt, c):
                    if c == 0:
                        state["vps"] = ps_proj.tile([128, 128], f32, tag="proj",
                                                    name="vps")
                    nc.tensor.matmul(
                        state["vps"][:],
                        lhsT=state["kvt"][:, c * 512 + tt * 128:
                                          c * 512 + (tt + 1) * 128],
                        rhs=wv_sb[:, c * 128:(c + 1) * 128],
                        start=(c == 0), stop=(c == 15))
                    if c == 15:
                        tok = tch * 4 + tt
                        for gl in range(2):
                            base = gl * 1040 + tok * 65
                            nc.vector.tensor_tensor(
                                vaug_sb[:, base:base + 64],
                                state["vps"][:, gl * 64:(gl + 1) * 64],
                                bvt_sb[:, gl * 64:(gl + 1) * 64], ALU.add)
                            nc.vector.memset(
                                vaug_sb[:, base + 64:base + 65], 1.0)
                for tt in range(4):
                    for c in range(0, 16, 4):
                        # 4 small matmuls per thunk (they are ~68ns each)
                        def v4(tt=tt, c0=c):
                            for c in range(c0, c0 + 4):
                                vmm(tt, c)
                        th.append(v4)
                return th

            def q_chain_thunks(qch):
                """Q projection for q chunk qch: DMA + 4 head-chunk chains."""
                th = []
                state = {}

                def dma():
                    xt = stream.tile([128, 16 * 512], bf16, tag="xs", name="xt")
                    for c in range(16):
                        nc.sync.dma_start(
                            out=xt[:, c * 512:(c + 1) * 512],
                            in_=xT_d[c * 128:(c + 1) * 128,
                                     qch * 512:(qch + 1) * 512])
                    state["xt"] = xt
                th.append(dma)

                def qmm(hc, c):
                    if c == 0:
                        state["qps"] = ps_proj.tile([128, 512], f32, tag="proj",
                                                    name="qps")
                    nc.tensor.matmul(
                        state["qps"][:],
                        lhsT=wq_sb[:, c * 512 + hc * 128:c * 512 + (hc + 1) * 128],
                        rhs=state["xt"][:, c * 512:(c + 1) * 512],
                        start=(c == 0), stop=(c == 15))
                    if c == 15:
                        nc.vector.tensor_scalar(
                            qt_sb[:, hc * 2048 + qch * 512:
                                  hc * 2048 + (qch + 1) * 512],
                            state["qps"][:], bq_sb[:, hc:hc + 1], None, ALU.add)
                for hc in range(4):
                    for c in range(16):
                        th.append(lambda hc=hc, c=c: qmm(hc, c))
                return th

            def outproj_thunks(jqb):
                """Out-projection for q block jqb (4 q-tiles x 4 col-chunks)."""
                th = []
                state = {}

                def omm(qt_i, cc, c):
                    if c == 0:
                        state["outp"] = ps_proj.tile([128, 512], f32, tag="proj",
                                                     name="outp")
                    nc.tensor.matmul(
                        state["outp"][:],
                        lhsT=ot_sb[:, c * 2048 + qt_i * 128:
                                   c * 2048 + (qt_i + 1) * 128],
                        rhs=wo_sb[:, c * 2048 + cc * 512:c * 2048 + (cc + 1) * 512],
                        start=(c == 0), stop=(c == 3))
                    if c == 3:
                        osb = stream.tile([128, 512], f32, tag="osb", name="osb")
                        # scalar engine: idle outside the exp stream
                        nc.scalar.activation(osb[:], state["outp"][:], AF.Copy)
                        nc.sync.dma_start(
                            out=out_d[qt_i * 128:(qt_i + 1) * 128,
                                      cc * 512:(cc + 1) * 512], in_=osb[:])
                for qt_i in range(jqb * 4, jqb * 4 + 4):
                    for cc in range(4):
                        for c in range(4):
                            th.append(lambda q=qt_i, cc=cc, c=c: omm(q, cc, c))
                return th

            # ---- filler queue machinery ----
            fillers = []
            fpos = [0]

            def pop_filler(n=1):
                while n > 0 and fpos[0] < len(fillers):
                    fillers[fpos[0]]()
                    fpos[0] += 1
                    n -= 1

            def drain_fillers_through(idx):
                while fpos[0] <= idx:
                    fillers[fpos[0]]()
                    fpos[0] += 1

            # ---- attention for one (head, q-chunk) with 2-deep QK pipeline ----
            def attention(h, jq):
                gl = h // 4
                hc, hr = h % 4, gl * 64
                nkc = 4 * jq + 4
                qbase = hc * 2048 + jq * 512
                ops = ps_o.tile([65, 512], f32, tag="ops", name="ops")
                sps_t = {}
                pt_t = {}

                def emit_qk(kci):
                    m = max(0, kci * 128 - jq * 512)
                    sps = ps_s.tile([128, 512], f32, tag="sps", name="sps")
                    nc.tensor.matmul(
                        sps[:, m:512],
                        lhsT=kt_sb[gl * 64:(gl + 1) * 64,
                                   kci * 128:(kci + 1) * 128],
                        rhs=qt_sb[hr:hr + 64, qbase + m:qbase + 512],
                        start=True, stop=True)
                    sps_t[kci] = (sps, m)

                def emit_exp(kci):
                    sps, m = sps_t.pop(kci)
                    pt = probs_pool.tile([128, 512], bf16, tag="pt", name="pt")
                    nc.scalar.activation(pt[:, m:512], sps[:, m:512],
                                         AF.Exp, scale=SCALE)
                    if kci >= 4 * jq:   # diagonal chunk -> mask
                        nc.vector.tensor_tensor(
                            pt[:, m:512], pt[:, m:512],
                            m0_sb[:, 512:1024 - m], ALU.mult)
                    pt_t[kci] = (pt, m)

                def emit_pv(kci):
                    pt, m = pt_t.pop(kci)
                    vbase = gl * 1040 + kci * 65
                    nc.tensor.matmul(
                        ops[:, m:512], lhsT=vaug_sb[:, vbase:vbase + 65],
                        rhs=pt[:, m:512],
                        start=(kci == 0), stop=(kci == nkc - 1))

                emit_qk(0)
                if nkc > 1:
                    emit_qk(1)
                for kci in range(nkc):
                    emit_exp(kci)
                    if kci + 2 < nkc:
                        emit_qk(kci + 2)
                    pop_filler(1)
                    emit_pv(kci)
                # normalize: 1/sums broadcast down partitions via ones-matmul
                rss = small.tile([1, 512], f32, tag="rss", name="rss")
                nc.vector.tensor_copy(rss[:], ops[64:65, :])
                rs = small.tile([1, 512], f32, tag="rs")
                nc.vector.reciprocal_approx_fast(rs[:], rss[:])
                rsb = small.tile([1, 512], bf16, tag="rsb")
                nc.vector.tensor_copy(rsb[:], rs[:])
                bps = ps_b.tile([64, 512], f32, tag="bps", name="bps")
                nc.tensor.matmul(bps[:], lhsT=ones_sb[:], rhs=rsb[:],
                                 start=True, stop=True)
                bsb = small.tile([64, 512], f32, tag="bsb", name="bsb")
                nc.vector.tensor_copy(bsb[:], bps[:])
                nc.vector.tensor_tensor(
                    ot_sb[hr:hr + 64, qbase:qbase + 512],
                    ops[0:64, :], bsb[:], ALU.mult)

            # ---- emission schedule ----
            # prologue: KV(0) + Q(0) emitted directly
            for t in kv_chain_thunks(0):
                t()
            for t in q_chain_thunks(0):
                t()
            # fillers, dependency-safe order; record end index of each group
            group_end = {}
            for name, th in [("kv1", kv_chain_thunks(1)),
                             ("kv2", kv_chain_thunks(2)),
                             ("kv3", kv_chain_thunks(3)),
                             ("q1", q_chain_thunks(1)),
                             ("q2", q_chain_thunks(2)),
                             ("q3", q_chain_thunks(3))]:
                fillers.extend(th)
                group_end[name] = len(fillers) - 1

            for jq in range(4):
                # producers attention(jq) needs must be emitted already
                if jq >= 1:
                    drain_fillers_through(group_end[f"kv{jq}"])
                    drain_fillers_through(group_end[f"q{jq}"])
                for h in range(HPC):
                    attention(h, jq)
                    pop_filler(2)
                # out-proj of this block becomes legal filler now
                fillers.extend(outproj_thunks(jq))
                group_end[f"op{jq}"] = len(fillers) - 1
            pop_filler(len(fillers))
    nc.finalize()
    return nc


def _get_nc():
    if "nc" not in _CACHE:
        _CACHE["nc"] = _build()
    return _CACHE["nc"]


def kernel(**inputs):
    out, _ = _run(inputs, trace=False)
    return out


def _run(inputs, trace=False):
    import ml_dtypes
    from concourse.bass_utils import run_bass_kernel_spmd

    x = np.asarray(inputs["x"], np.float32)
    kv = np.asarray(inputs["kv"], np.float32)
    Wq = np.asarray(inputs["Wq"], np.float32)
    bq = np.asarray(inputs["bq"], np.float32)
    Wk = np.asarray(inputs["Wk"], np.float32)
    bk = np.asarray(inputs["bk"], np.float32)
    Wv = np.asarray(inputs["Wv"], np.float32)
    bv = np.asarray(inputs["bv"], np.float32)
    Wo = np.asarray(inputs["Wo"], np.float32)
    bo = np.asarray(inputs["bo"], np.float32)

    bf = ml_dtypes.bfloat16
    M0 = (np.arange(1024)[None, :] >= (np.arange(128)[:, None] + 512)
          ).astype(bf)

    # head-dim permutation: chunk c = [local head c | local head 4+c]
    # so each head's Q rows sit at the partition half of its KV group.
    hperm = np.concatenate(
        [np.concatenate([np.arange(c * 64, c * 64 + 64),
                         np.arange((4 + c) * 64, (4 + c) * 64 + 64)])
         for c in range(4)])  # [512] permutation of local head dims

    in_maps = []
    for core in range(NCORES):
        b, t = core // 4, core % 4
        bv_sh = bv[t * 128:(t + 1) * 128]
        bvt = np.broadcast_to(bv_sh[None, :], (128, 128)).astype(np.float32)
        wq_sh = Wq[:, t * 512:(t + 1) * 512][:, hperm]
        wo_sh = Wo[t * 512:(t + 1) * 512, :][hperm, :]
        bq_sh = bq[t * 512:(t + 1) * 512][hperm]
        in_maps.append({
            "xT": np.ascontiguousarray(x[b].T).astype(bf),
            "kvT": np.ascontiguousarray(kv[b].T).astype(bf),
            "wq": wq_sh.astype(bf),
            "wk": Wk[:, t * 128:(t + 1) * 128].astype(bf),
            "wv": Wv[:, t * 128:(t + 1) * 128].astype(bf),
            "wo": np.ascontiguousarray(wo_sh).astype(bf),
            "bq": np.ascontiguousarray(bq_sh.reshape(4, 128).T),
            "bk": bk[t * 128:(t + 1) * 128].reshape(128, 1).copy(),
            "bvt": np.ascontiguousarray(bvt),
            "m0": M0,
        })

    nc = _get_nc()
    res = run_bass_kernel_spmd(nc, in_maps, core_ids=list(range(NCORES)),
                               trace=trace)
    parts = [np.asarray(res.results[i]["out"], np.float32)
             for i in range(NCORES)]
    out = np.stack([parts[0] + parts[1] + parts[2] + parts[3],
                    parts[4] + parts[5] + parts[6] + parts[7]])
    out += bo[None, None, :]
    return out.astype(np.float32), res


# revision 19
# speedup vs baseline: 1.6750x; 1.0040x over previous
"""GQA attention kernel for 8 TRN2 NeuronCores.

Sharding: data-parallel over batch (B=2) x tensor-parallel over heads (4-way).
Core i handles batch i//4 and head-shard i%4 (8 query heads = 2 KV groups).
Out-proj is row-sharded; the 4 partial [S,D] outputs per batch are summed on
the host (cheap unshard step), bo added once.

Device kernel (per core, all bf16 matmuls, f32 PSUM):
  QT = Wq_sh.T @ xT          [512, S]   (x pre-transposed on host)
  KT = Wk_sh.T @ kvT         [128, S]
  V  = kvT.T  @ Wv_sh        [S, 128] -> per-group V_aug [S, 64+1] (ones col)
  per (head, q-chunk 512): scores^T chunks [128 keys, 512 q] -> exp (no max
  subtraction; scores are O(1)) -> causal mask via sliding window of a
  precomputed [128,1024] 0/1 mask -> PV accumulate with ones-row giving
  softmax sums in row 64 -> normalize via reciprocal + ones-outer-product
  broadcast -> OT [512, S] -> out_partial = OT.T @ Wo_sh  [S, D] f32.
"""

import numpy as np

B, S, D = 2, 2048, 2048
H, G, HD, GS = 32, 8, 64, 4
HPC = 8     # query heads per core
GPC = 2     # kv groups per core
NCORES = 8
SCALE = 0.125  # 1/sqrt(64)

_CACHE = {}


def _build():
    import concourse.bass as bass
    import concourse.tile as tile
    from concourse import bacc, mybir

    f32 = mybir.dt.float32
    bf16 = mybir.dt.bfloat16
    AF = mybir.ActivationFunctionType
    ALU = mybir.AluOpType

    nc = bacc.Bacc("TRN2", target_bir_lowering=False, debug=False,
                   num_devices=NCORES)

    xT_d = nc.declare_dram_parameter("xT", [D, S], bf16, isOutput=False)
    kvT_d = nc.declare_dram_parameter("kvT", [D, S], bf16, isOutput=False)
    wq_d = nc.declare_dram_parameter("wq", [D, 512], bf16, isOutput=False)
    wk_d = nc.declare_dram_parameter("wk", [D, 128], bf16, isOutput=False)
    wv_d = nc.declare_dram_parameter("wv", [D, 128], bf16, isOutput=False)
    wo_d = nc.declare_dram_parameter("wo", [512, D], bf16, isOutput=False)
    bq_d = nc.declare_dram_parameter("bq", [128, 4], f32, isOutput=False)
    bk_d = nc.declare_dram_parameter("bk", [128, 1], f32, isOutput=False)
    bvt_d = nc.declare_dram_parameter("bvt", [128, 2 * 64], f32, isOutput=False)
    m0_d = nc.declare_dram_parameter("m0", [128, 1024], bf16, isOutput=False)
    out_d = nc.declare_dram_parameter("out", [S, D], f32, isOutput=True)

    with tile.TileContext(nc) as tc:
        with (
            tc.tile_pool(name="persist", bufs=1) as persist,
            tc.tile_pool(name="stream", bufs=3) as stream,
            tc.tile_pool(name="small", bufs=3) as small,
            tc.tile_pool(name="probs", bufs=6) as probs_pool,
            tc.tile_pool(name="ps_s", bufs=3, space="PSUM") as ps_s,
            tc.tile_pool(name="ps_proj", bufs=2, space="PSUM") as ps_proj,
            tc.tile_pool(name="ps_o", bufs=2, space="PSUM") as ps_o,
            tc.tile_pool(name="ps_b", bufs=1, space="PSUM") as ps_b,
        ):
            # ---- resident weight tiles (DMAs emitted in compute order) ----
            wq_sb = persist.tile([128, 16 * 512], bf16, tag="wq")   # chunk c at c*512
            wk_sb = persist.tile([128, 16 * 128], bf16, tag="wk")
            wv_sb = persist.tile([128, 16 * 128], bf16, tag="wv")
            wo_sb = persist.tile([128, 4 * 2048], bf16, tag="wo")
            m0_sb = persist.tile([128, 1024], bf16, tag="m0")
            bq_sb = persist.tile([128, 4], f32, tag="bq")
            bk_sb = persist.tile([128, 1], f32, tag="bk")
            bvt_sb = persist.tile([128, 2 * 64], f32, tag="bvt")
            ones_sb = persist.tile([1, 64], bf16, tag="ones")

            # loads needed by the kv0 chain, first
            nc.sync.dma_start(out=bk_sb[:], in_=bk_d[:, :])
            nc.sync.dma_start(out=bvt_sb[:], in_=bvt_d[:, :])
            for c in range(16):
                nc.sync.dma_start(out=wk_sb[:, c * 128:(c + 1) * 128],
                                  in_=wk_d[c * 128:(c + 1) * 128, :])
            for c in range(16):
                nc.sync.dma_start(out=wv_sb[:, c * 128:(c + 1) * 128],
                                  in_=wv_d[c * 128:(c + 1) * 128, :])

            # ---- resident projection outputs ----
            qt_sb = persist.tile([128, 4 * 2048], bf16, tag="qt")   # chunk hc at hc*2048
            kt_sb = persist.tile([128, S], bf16, tag="kt")
            vaug_sb = persist.tile([128, 2 * 16 * 65], bf16, tag="vaug")  # [gl*1040+tok*65]
            ot_sb = persist.tile([128, 4 * 2048], bf16, tag="ot")

            # ---- chain emitters (as thunk lists for PE-filler interleave) ----
            def kv_chain_thunks(tch):
                """K/V projection for kv token chunk tch: DMA + KT + V."""
                th = []
                state = {}

                def dma():
                    kvt = stream.tile([128, 16 * 512], bf16, tag="xs", name="kvt")
                    for c in range(16):
                        nc.sync.dma_start(
                            out=kvt[:, c * 512:(c + 1) * 512],
                            in_=kvT_d[c * 128:(c + 1) * 128,
                                      tch * 512:(tch + 1) * 512])
                    state["kvt"] = kvt
                    state["kps"] = ps_proj.tile([128, 512], f32, tag="proj",
                                                name="kps")
                th.append(dma)

                def kmm(c):
                    nc.tensor.matmul(
                        state["kps"][:], lhsT=wk_sb[:, c * 128:(c + 1) * 128],
                        rhs=state["kvt"][:, c * 512:(c + 1) * 512],
                        start=(c == 0), stop=(c == 15))
                    if c == 15:
                        nc.vector.tensor_scalar(
                            kt_sb[:, tch * 512:(tch + 1) * 512], state["kps"][:],
                            bk_sb[:, 0:1], None, ALU.add)
                for c in range(16):
                    th.append(lambda c=c: kmm(c))

                def vmm(tt, c):
                    if c == 0:
                        state["vps"] = ps_proj.tile([128, 128], f32, tag="proj",
                                                    name="vps")
                    nc.tensor.matmul(
                        state["vps"][:],
                        lhsT=state["kvt"][:, c * 512 + tt * 128:
                                          c * 512 + (tt + 1) * 128],
                        rhs=wv_sb[:, c * 128:(c + 1) * 128],
                        start=(c == 0), stop=(c == 15))
                    if c == 15:
                        tok = tch * 4 + tt
                        for gl in range(2):
                            base = gl * 1040 + tok * 65
                            nc.vector.tensor_tensor(
                                vaug_sb[:, base:base + 64],
                                state["vps"][:, gl * 64:(gl + 1) * 64],
                                bvt_sb[:, gl * 64:(gl + 1) * 64], ALU.add)
                            nc.vector.memset(
                                vaug_sb[:, base + 64:base + 65], 1.0)
                for tt in range(4):
                    for c in range(0, 16, 4):
                        # 4 small matmuls per thunk (they are ~68ns each)
                        def v4(tt=tt, c0=c):
                            for c in range(c0, c0 + 4):
                                vmm(tt, c)
                        th.append(v4)
                return th

            def q_chain_thunks(qch):
                """Q projection for q chunk qch: DMA + 4 head-chunk chains."""
                th = []
                state = {}

                def dma():
                    xt = stream.tile([128, 16 * 512], bf16, tag="xs", name="xt")
                    for c in range(16):
                        nc.sync.dma_start(
                            out=xt[:, c * 512:(c + 1) * 512],
                            in_=xT_d[c * 128:(c + 1) * 128,
                                     qch * 512:(qch + 1) * 512])
                    state["xt"] = xt
                th.append(dma)

                def qmm(hc, c):
                    if c == 0:
                        state["qps"] = ps_proj.tile([128, 512], f32, tag="proj",
                                                    name="qps")
                    nc.tensor.matmul(
                        state["qps"][:],
                        lhsT=wq_sb[:, c * 512 + hc * 128:c * 512 + (hc + 1) * 128],
                        rhs=state["xt"][:, c * 512:(c + 1) * 512],
                        start=(c == 0), stop=(c == 15))
                    if c == 15:
                        nc.vector.tensor_scalar(
                            qt_sb[:, hc * 2048 + qch * 512:
                                  hc * 2048 + (qch + 1) * 512],
                            state["qps"][:], bq_sb[:, hc:hc + 1], None, ALU.add)
                for hc in range(4):
                    for c in range(16):
                        th.append(lambda hc=hc, c=c: qmm(hc, c))
                return th

            def outproj_thunks(jqb):
                """Out-projection for q block jqb (4 q-tiles x 4 col-chunks)."""
                th = []
                state = {}

                def omm(qt_i, cc, c):
                    if c == 0:
                        state["outp"] = ps_proj.tile([128, 512], f32, tag="proj",
                                                     name="outp")
                    nc.tensor.matmul(
                        state["outp"][:],
                        lhsT=ot_sb[:, c * 2048 + qt_i * 128:
                                   c * 2048 + (qt_i + 1) * 128],
                        rhs=wo_sb[:, c * 2048 + cc * 512:c * 2048 + (cc + 1) * 512],
                        start=(c == 0), stop=(c == 3))
                    if c == 3:
                        osb = stream.tile([128, 512], f32, tag="osb", name="osb")
                        # scalar engine: idle outside the exp stream
                        nc.scalar.activation(osb[:], state["outp"][:], AF.Copy)
                        nc.sync.dma_start(
                            out=out_d[qt_i * 128:(qt_i + 1) * 128,
                                      cc * 512:(cc + 1) * 512], in_=osb[:])
                for qt_i in range(jqb * 4, jqb * 4 + 4):
                    for cc in range(4):
                        for c in range(4):
                            th.append(lambda q=qt_i, cc=cc, c=c: omm(q, cc, c))
                return th

            # ---- filler queue machinery ----
            fillers = []
            fpos = [0]

            def pop_filler(n=1):
                while n > 0 and fpos[0] < len(fillers):
                    fillers[fpos[0]]()
                    fpos[0] += 1
                    n -= 1

            def drain_fillers_through(idx):
                while fpos[0] <= idx:
                    fillers[fpos[0]]()
                    fpos[0] += 1

            # ---- attention for one (head, q-chunk) with 2-deep QK pipeline ----
            def attention(h, jq):
                gl = h // 4
                hc, hr = h % 4, gl * 64
                nkc = 4 * jq + 4
                qbase = hc * 2048 + jq * 512
                ops = ps_o.tile([65, 512], f32, tag="ops", name="ops")
                sps_t = {}
                pt_t = {}

                def emit_qk(kci):
                    m = max(0, kci * 128 - jq * 512)
                    sps = ps_s.tile([128, 512], f32, tag="sps", name="sps")
                    nc.tensor.matmul(
                        sps[:, m:512],
                        lhsT=kt_sb[gl * 64:(gl + 1) * 64,
                                   kci * 128:(kci + 1) * 128],
                        rhs=qt_sb[hr:hr + 64, qbase + m:qbase + 512],
                        start=True, stop=True)
                    sps_t[kci] = (sps, m)

                def emit_exp(kci):
                    sps, m = sps_t.pop(kci)
                    pt = probs_pool.tile([128, 512], bf16, tag="pt", name="pt")
                    nc.scalar.activation(pt[:, m:512], sps[:, m:512],
                                         AF.Exp, scale=SCALE)
                    if kci >= 4 * jq:   # diagonal chunk -> mask
                        nc.vector.tensor_tensor(
                            pt[:, m:512], pt[:, m:512],
                            m0_sb[:, 512:1024 - m], ALU.mult)
                    pt_t[kci] = (pt, m)

                def emit_pv(kci):
                    pt, m = pt_t.pop(kci)
                    vbase = gl * 1040 + kci * 65
                    nc.tensor.matmul(
                        ops[:, m:512], lhsT=vaug_sb[:, vbase:vbase + 65],
                        rhs=pt[:, m:512],
                        start=(kci == 0), stop=(kci == nkc - 1))

                emit_qk(0)
                if nkc > 1:
                    emit_qk(1)
                for kci in range(nkc):
                    emit_exp(kci)
                    if kci + 2 < nkc:
                        emit_qk(kci + 2)
                    pop_filler(1)
                    emit_pv(kci)
                # normalize: 1/sums broadcast down partitions via ones-matmul
                rss = small.tile([1, 512], f32, tag="rss", name="rss")
                nc.vector.tensor_copy(rss[:], ops[64:65, :])
                rs = small.tile([1, 512], f32, tag="rs")
                nc.vector.reciprocal_approx_fast(rs[:], rss[:])
                rsb = small.tile([1, 512], bf16, tag="rsb")
                nc.vector.tensor_copy(rsb[:], rs[:])
                bps = ps_b.tile([64, 512], f32, tag="bps", name="bps")
                nc.tensor.matmul(bps[:], lhsT=ones_sb[:], rhs=rsb[:],
                                 start=True, stop=True)
                bsb = small.tile([64, 512], f32, tag="bsb", name="bsb")
                nc.vector.tensor_copy(bsb[:], bps[:])
                nc.vector.tensor_tensor(
                    ot_sb[hr:hr + 64, qbase:qbase + 512],
                    ops[0:64, :], bsb[:], ALU.mult)

            # ---- emission schedule ----
            # prologue: KV(0) + Q(0) emitted directly; remaining weight DMAs
            # stream in behind compute
            for t in kv_chain_thunks(0):
                t()
            nc.sync.dma_start(out=bq_sb[:], in_=bq_d[:, :])
            for c in range(16):
                nc.sync.dma_start(out=wq_sb[:, c * 512:(c + 1) * 512],
                                  in_=wq_d[c * 128:(c + 1) * 128, :])
            nc.sync.dma_start(out=m0_sb[:], in_=m0_d[:, :])
            nc.vector.memset(ones_sb[:], 1.0)
            for t in q_chain_thunks(0):
                t()
            for c in range(4):
                nc.sync.dma_start(out=wo_sb[:, c * 2048:(c + 1) * 2048],
                                  in_=wo_d[c * 128:(c + 1) * 128, :])
            # fillers, dependency-safe order; record end index of each group
            group_end = {}
            for name, th in [("kv1", kv_chain_thunks(1)),
                             ("kv2", kv_chain_thunks(2)),
                             ("kv3", kv_chain_thunks(3)),
                             ("q1", q_chain_thunks(1)),
                             ("q2", q_chain_thunks(2)),
                             ("q3", q_chain_thunks(3))]:
                fillers.extend(th)
                group_end[name] = len(fillers) - 1

            for jq in range(4):
                # producers attention(jq) needs must be emitted already
                if jq >= 1:
                    drain_fillers_through(group_end[f"kv{jq}"])
                    drain_fillers_through(group_end[f"q{jq}"])
                for h in range(HPC):
                    attention(h, jq)
                    pop_filler(2)
                # out-proj of this block becomes legal filler now
                fillers.extend(outproj_thunks(jq))
                group_end[f"op{jq}"] = len(fillers) - 1
            pop_filler(len(fillers))
    nc.finalize()
    return nc


def _get_nc():
    if "nc" not in _CACHE:
        _CACHE["nc"] = _build()
    return _CACHE["nc"]


def kernel(**inputs):
    out, _ = _run(inputs, trace=False)
    return out


def _run(inputs, trace=False):
    import ml_dtypes
    from concourse.bass_utils import run_bass_kernel_spmd

    x = np.asarray(inputs["x"], np.float32)
    kv = np.asarray(inputs["kv"], np.float32)
    Wq = np.asarray(inputs["Wq"], np.float32)
    bq = np.asarray(inputs["bq"], np.float32)
    Wk = np.asarray(inputs["Wk"], np.float32)
    bk = np.asarray(inputs["bk"], np.float32)
    Wv = np.asarray(inputs["Wv"], np.float32)
    bv = np.asarray(inputs["bv"], np.float32)
    Wo = np.asarray(inputs["Wo"], np.float32)
    bo = np.asarray(inputs["bo"], np.float32)

    bf = ml_dtypes.bfloat16
    M0 = (np.arange(1024)[None, :] >= (np.arange(128)[:, None] + 512)
          ).astype(bf)

    # head-dim permutation: chunk c = [local head c | local head 4+c]
    # so each head's Q rows sit at the partition half of its KV group.
    hperm = np.concatenate(
        [np.concatenate([np.arange(c * 64, c * 64 + 64),
                         np.arange((4 + c) * 64, (4 + c) * 64 + 64)])
         for c in range(4)])  # [512] permutation of local head dims

    in_maps = []
    for core in range(NCORES):
        b, t = core // 4, core % 4
        bv_sh = bv[t * 128:(t + 1) * 128]
        bvt = np.broadcast_to(bv_sh[None, :], (128, 128)).astype(np.float32)
        wq_sh = Wq[:, t * 512:(t + 1) * 512][:, hperm]
        wo_sh = Wo[t * 512:(t + 1) * 512, :][hperm, :]
        bq_sh = bq[t * 512:(t + 1) * 512][hperm]
        in_maps.append({
            "xT": np.ascontiguousarray(x[b].T).astype(bf),
            "kvT": np.ascontiguousarray(kv[b].T).astype(bf),
            "wq": wq_sh.astype(bf),
            "wk": Wk[:, t * 128:(t + 1) * 128].astype(bf),
            "wv": Wv[:, t * 128:(t + 1) * 128].astype(bf),
            "wo": np.ascontiguousarray(wo_sh).astype(bf),
            "bq": np.ascontiguousarray(bq_sh.reshape(4, 128).T),
            "bk": bk[t * 128:(t + 1) * 128].reshape(128, 1).copy(),
            "bvt": np.ascontiguousarray(bvt),
            "m0": M0,
        })

    nc = _get_nc()
    res = run_bass_kernel_spmd(nc, in_maps, core_ids=list(range(NCORES)),
                               trace=trace)
    parts = [np.asarray(res.results[i]["out"], np.float32)
             for i in range(NCORES)]
    out = np.stack([parts[0] + parts[1] + parts[2] + parts[3],
                    parts[4] + parts[5] + parts[6] + parts[7]])
    out += bo[None, None, :]
    return out.astype(np.float32), res


# revision 20
# speedup vs baseline: 1.7921x; 1.0699x over previous
"""GQA attention kernel for 8 TRN2 NeuronCores.

Sharding: data-parallel over batch (B=2) x tensor-parallel over heads (4-way).
Core i handles batch i//4 and head-shard i%4 (8 query heads = 2 KV groups).
Out-proj is row-sharded; the 4 partial [S,D] outputs per batch are summed on
the host (cheap unshard step), bo added once.

Device kernel (per core, all bf16 matmuls, f32 PSUM):
  QT = Wq_sh.T @ xT          [512, S]   (x pre-transposed on host)
  KT = Wk_sh.T @ kvT         [128, S]
  V  = kvT.T  @ Wv_sh        [S, 128] -> per-group V_aug [S, 64+1] (ones col)
  per (head, q-chunk 512): scores^T chunks [128 keys, 512 q] -> exp (no max
  subtraction; scores are O(1)) -> causal mask via sliding window of a
  precomputed [128,1024] 0/1 mask -> PV accumulate with ones-row giving
  softmax sums in row 64 -> normalize via reciprocal + ones-outer-product
  broadcast -> OT [512, S] -> out_partial = OT.T @ Wo_sh  [S, D] f32.
"""

import numpy as np

B, S, D = 2, 2048, 2048
H, G, HD, GS = 32, 8, 64, 4
HPC = 8     # query heads per core
GPC = 2     # kv groups per core
NCORES = 8
SCALE = 0.125  # 1/sqrt(64)

_CACHE = {}


def _build():
    import concourse.bass as bass
    import concourse.tile as tile
    from concourse import bacc, mybir

    f32 = mybir.dt.float32
    bf16 = mybir.dt.bfloat16
    AF = mybir.ActivationFunctionType
    ALU = mybir.AluOpType

    nc = bacc.Bacc("TRN2", target_bir_lowering=False, debug=False,
                   num_devices=NCORES)

    xT_d = nc.declare_dram_parameter("xT", [D, S], bf16, isOutput=False)
    kvT_d = nc.declare_dram_parameter("kvT", [D, S], bf16, isOutput=False)
    wq_d = nc.declare_dram_parameter("wq", [D, 512], bf16, isOutput=False)
    wk_d = nc.declare_dram_parameter("wk", [D, 128], bf16, isOutput=False)
    wv_d = nc.declare_dram_parameter("wv", [D, 128], bf16, isOutput=False)
    wo_d = nc.declare_dram_parameter("wo", [512, D], bf16, isOutput=False)
    bq_d = nc.declare_dram_parameter("bq", [128, 4], f32, isOutput=False)
    bk_d = nc.declare_dram_parameter("bk", [128, 1], f32, isOutput=False)
    bvt_d = nc.declare_dram_parameter("bvt", [128, 2 * 64], f32, isOutput=False)
    m0_d = nc.declare_dram_parameter("m0", [128, 1024], bf16, isOutput=False)
    out_d = nc.declare_dram_parameter("out", [S, D], f32, isOutput=True)

    with tile.TileContext(nc) as tc:
        with (
            tc.tile_pool(name="persist", bufs=1) as persist,
            tc.tile_pool(name="stream", bufs=3) as stream,
            tc.tile_pool(name="small", bufs=3) as small,
            tc.tile_pool(name="probs", bufs=6) as probs_pool,
            tc.tile_pool(name="ps_s", bufs=3, space="PSUM") as ps_s,
            tc.tile_pool(name="ps_proj", bufs=2, space="PSUM") as ps_proj,
            tc.tile_pool(name="ps_o", bufs=2, space="PSUM") as ps_o,
            tc.tile_pool(name="ps_b", bufs=1, space="PSUM") as ps_b,
        ):
            # ---- resident weight tiles (DMAs emitted in compute order) ----
            wq_sb = persist.tile([128, 16 * 512], bf16, tag="wq")   # chunk c at c*512
            wk_sb = persist.tile([128, 16 * 128], bf16, tag="wk")
            wv_sb = persist.tile([128, 16 * 128], bf16, tag="wv")
            wo_sb = persist.tile([128, 4 * 2048], bf16, tag="wo")
            m0_sb = persist.tile([128, 1024], bf16, tag="m0")
            bq_sb = persist.tile([128, 4], f32, tag="bq")
            bk_sb = persist.tile([128, 1], f32, tag="bk")
            bvt_sb = persist.tile([128, 2 * 64], f32, tag="bvt")
            ones_sb = persist.tile([1, 64], bf16, tag="ones")

            # loads needed by the kv0 chain, first
            nc.sync.dma_start(out=bk_sb[:], in_=bk_d[:, :])
            nc.sync.dma_start(out=bvt_sb[:], in_=bvt_d[:, :])
            nc.sync.dma_start(
                out=wk_sb.rearrange("p (c j) -> p c j", j=128),
                in_=wk_d.rearrange("(c p) j -> p c j", p=128))
            nc.sync.dma_start(
                out=wv_sb.rearrange("p (c j) -> p c j", j=128),
                in_=wv_d.rearrange("(c p) j -> p c j", p=128))

            # ---- resident projection outputs ----
            qt_sb = persist.tile([128, 4 * 2048], bf16, tag="qt")   # chunk hc at hc*2048
            kt_sb = persist.tile([128, S], bf16, tag="kt")
            vaug_sb = persist.tile([128, 2 * 16 * 65], bf16, tag="vaug")  # [gl*1040+tok*65]
            ot_sb = persist.tile([128, 4 * 2048], bf16, tag="ot")

            # ---- chain emitters (as thunk lists for PE-filler interleave) ----
            def kv_chain_thunks(tch):
                """K/V projection for kv token chunk tch: DMA + KT + V."""
                th = []
                state = {}

                def dma():
                    kvt = stream.tile([128, 16 * 512], bf16, tag="xs", name="kvt")
                    nc.sync.dma_start(
                        out=kvt.rearrange("p (c j) -> p c j", j=512),
                        in_=kvT_d[:, tch * 512:(tch + 1) * 512]
                        .rearrange("(c p) j -> p c j", p=128))
                    state["kvt"] = kvt
                    state["kps"] = ps_proj.tile([128, 512], f32, tag="proj",
                                                name="kps")
                th.append(dma)

                def kmm(c):
                    nc.tensor.matmul(
                        state["kps"][:], lhsT=wk_sb[:, c * 128:(c + 1) * 128],
                        rhs=state["kvt"][:, c * 512:(c + 1) * 512],
                        start=(c == 0), stop=(c == 15))
                    if c == 15:
                        nc.vector.tensor_scalar(
                            kt_sb[:, tch * 512:(tch + 1) * 512], state["kps"][:],
                            bk_sb[:, 0:1], None, ALU.add)
                for c in range(16):
                    th.append(lambda c=c: kmm(c))

                def vmm(tt, c):
                    if c == 0:
                        state["vps"] = ps_proj.tile([128, 128], f32, tag="proj",
                                                    name="vps")
                    nc.tensor.matmul(
                        state["vps"][:],
                        lhsT=state["kvt"][:, c * 512 + tt * 128:
                                          c * 512 + (tt + 1) * 128],
                        rhs=wv_sb[:, c * 128:(c + 1) * 128],
                        start=(c == 0), stop=(c == 15))
                    if c == 15:
                        tok = tch * 4 + tt
                        for gl in range(2):
                            base = gl * 1040 + tok * 65
                            nc.vector.tensor_tensor(
                                vaug_sb[:, base:base + 64],
                                state["vps"][:, gl * 64:(gl + 1) * 64],
                                bvt_sb[:, gl * 64:(gl + 1) * 64], ALU.add)
                            nc.vector.memset(
                                vaug_sb[:, base + 64:base + 65], 1.0)
                for tt in range(4):
                    for c in range(0, 16, 4):
                        # 4 small matmuls per thunk (they are ~68ns each)
                        def v4(tt=tt, c0=c):
                            for c in range(c0, c0 + 4):
                                vmm(tt, c)
                        th.append(v4)
                return th

            def q_chain_thunks(qch):
                """Q projection for q chunk qch: DMA + 4 head-chunk chains."""
                th = []
                state = {}

                def dma():
                    xt = stream.tile([128, 16 * 512], bf16, tag="xs", name="xt")
                    nc.sync.dma_start(
                        out=xt.rearrange("p (c j) -> p c j", j=512),
                        in_=xT_d[:, qch * 512:(qch + 1) * 512]
                        .rearrange("(c p) j -> p c j", p=128))
                    state["xt"] = xt
                th.append(dma)

                def qmm(hc, c):
                    if c == 0:
                        state["qps"] = ps_proj.tile([128, 512], f32, tag="proj",
                                                    name="qps")
                    nc.tensor.matmul(
                        state["qps"][:],
                        lhsT=wq_sb[:, c * 512 + hc * 128:c * 512 + (hc + 1) * 128],
                        rhs=state["xt"][:, c * 512:(c + 1) * 512],
                        start=(c == 0), stop=(c == 15))
                    if c == 15:
                        nc.vector.tensor_scalar(
                            qt_sb[:, hc * 2048 + qch * 512:
                                  hc * 2048 + (qch + 1) * 512],
                            state["qps"][:], bq_sb[:, hc:hc + 1], None, ALU.add)
                for hc in range(4):
                    for c in range(16):
                        th.append(lambda hc=hc, c=c: qmm(hc, c))
                return th

            def outproj_thunks(jqb):
                """Out-projection for q block jqb (4 q-tiles x 4 col-chunks)."""
                th = []
                state = {}

                def omm(qt_i, cc, c):
                    if c == 0:
                        state["outp"] = ps_proj.tile([128, 512], f32, tag="proj",
                                                     name="outp")
                    nc.tensor.matmul(
                        state["outp"][:],
                        lhsT=ot_sb[:, c * 2048 + qt_i * 128:
                                   c * 2048 + (qt_i + 1) * 128],
                        rhs=wo_sb[:, c * 2048 + cc * 512:c * 2048 + (cc + 1) * 512],
                        start=(c == 0), stop=(c == 3))
                    if c == 3:
                        if cc == 0:
                            state["osb"] = stream.tile([128, 2048], f32,
                                                       tag="osb", name="osb")
                        # scalar engine: idle outside the exp stream
                        nc.scalar.activation(
                            state["osb"][:, cc * 512:(cc + 1) * 512],
                            state["outp"][:], AF.Copy)
                        if cc == 3:
                            nc.sync.dma_start(
                                out=out_d[qt_i * 128:(qt_i + 1) * 128, :],
                                in_=state["osb"][:])
                for qt_i in range(jqb * 4, jqb * 4 + 4):
                    for cc in range(4):
                        for c in range(4):
                            th.append(lambda q=qt_i, cc=cc, c=c: omm(q, cc, c))
                return th

            # ---- filler queue machinery ----
            fillers = []
            fpos = [0]

            def pop_filler(n=1):
                while n > 0 and fpos[0] < len(fillers):
                    fillers[fpos[0]]()
                    fpos[0] += 1
                    n -= 1

            def drain_fillers_through(idx):
                while fpos[0] <= idx:
                    fillers[fpos[0]]()
                    fpos[0] += 1

            # ---- attention for one (head, q-chunk) with 2-deep QK pipeline ----
            def attention(h, jq):
                gl = h // 4
                hc, hr = h % 4, gl * 64
                nkc = 4 * jq + 4
                qbase = hc * 2048 + jq * 512
                ops = ps_o.tile([65, 512], f32, tag="ops", name="ops")
                sps_t = {}
                pt_t = {}

                def emit_qk(kci):
                    m = max(0, kci * 128 - jq * 512)
                    sps = ps_s.tile([128, 512], f32, tag="sps", name="sps")
                    nc.tensor.matmul(
                        sps[:, m:512],
                        lhsT=kt_sb[gl * 64:(gl + 1) * 64,
                                   kci * 128:(kci + 1) * 128],
                        rhs=qt_sb[hr:hr + 64, qbase + m:qbase + 512],
                        start=True, stop=True)
                    sps_t[kci] = (sps, m)

                def emit_exp(kci):
                    sps, m = sps_t.pop(kci)
                    pt = probs_pool.tile([128, 512], bf16, tag="pt", name="pt")
                    nc.scalar.activation(pt[:, m:512], sps[:, m:512],
                                         AF.Exp, scale=SCALE)
                    if kci >= 4 * jq:   # diagonal chunk -> mask
                        nc.vector.tensor_tensor(
                            pt[:, m:512], pt[:, m:512],
                            m0_sb[:, 512:1024 - m], ALU.mult)
                    pt_t[kci] = (pt, m)

                def emit_pv(kci):
                    pt, m = pt_t.pop(kci)
                    vbase = gl * 1040 + kci * 65
                    nc.tensor.matmul(
                        ops[:, m:512], lhsT=vaug_sb[:, vbase:vbase + 65],
                        rhs=pt[:, m:512],
                        start=(kci == 0), stop=(kci == nkc - 1))

                emit_qk(0)
                if nkc > 1:
                    emit_qk(1)
                for kci in range(nkc):
                    emit_exp(kci)
                    if kci + 2 < nkc:
                        emit_qk(kci + 2)
                    pop_filler(1)
                    emit_pv(kci)
                # normalize: 1/sums broadcast down partitions via ones-matmul
                rss = small.tile([1, 512], f32, tag="rss", name="rss")
                nc.vector.tensor_copy(rss[:], ops[64:65, :])
                rs = small.tile([1, 512], f32, tag="rs")
                nc.vector.reciprocal_approx_fast(rs[:], rss[:])
                rsb = small.tile([1, 512], bf16, tag="rsb")
                nc.vector.tensor_copy(rsb[:], rs[:])
                bps = ps_b.tile([64, 512], f32, tag="bps", name="bps")
                nc.tensor.matmul(bps[:], lhsT=ones_sb[:], rhs=rsb[:],
                                 start=True, stop=True)
                bsb = small.tile([64, 512], f32, tag="bsb", name="bsb")
                nc.vector.tensor_copy(bsb[:], bps[:])
                nc.vector.tensor_tensor(
                    ot_sb[hr:hr + 64, qbase:qbase + 512],
                    ops[0:64, :], bsb[:], ALU.mult)

            # ---- emission schedule ----
            # prologue: KV(0) + Q(0) emitted directly; remaining weight DMAs
            # stream in behind compute
            for t in kv_chain_thunks(0):
                t()
            nc.sync.dma_start(out=bq_sb[:], in_=bq_d[:, :])
            nc.sync.dma_start(
                out=wq_sb.rearrange("p (c j) -> p c j", j=512),
                in_=wq_d.rearrange("(c p) j -> p c j", p=128))
            nc.sync.dma_start(out=m0_sb[:], in_=m0_d[:, :])
            nc.vector.memset(ones_sb[:], 1.0)
            for t in q_chain_thunks(0):
                t()
            nc.sync.dma_start(
                out=wo_sb.rearrange("p (c j) -> p c j", j=2048),
                in_=wo_d.rearrange("(c p) j -> p c j", p=128))
            # fillers, dependency-safe order; record end index of each group
            group_end = {}
            for name, th in [("kv1", kv_chain_thunks(1)),
                             ("kv2", kv_chain_thunks(2)),
                             ("kv3", kv_chain_thunks(3)),
                             ("q1", q_chain_thunks(1)),
                             ("q2", q_chain_thunks(2)),
                             ("q3", q_chain_thunks(3))]:
                fillers.extend(th)
                group_end[name] = len(fillers) - 1

            for jq in range(4):
                # producers attention(jq) needs must be emitted already
                if jq >= 1:
                    drain_fillers_through(group_end[f"kv{jq}"])
                    drain_fillers_through(group_end[f"q{jq}"])
                for h in range(HPC):
                    attention(h, jq)
                    pop_filler(2)
                # out-proj of this block becomes legal filler now
                fillers.extend(outproj_thunks(jq))
                group_end[f"op{jq}"] = len(fillers) - 1
            pop_filler(len(fillers))
    nc.finalize()
    return nc


def _get_nc():
    if "nc" not in _CACHE:
        _CACHE["nc"] = _build()
    return _CACHE["nc"]


def kernel(**inputs):
    out, _ = _run(inputs, trace=False)
    return out


def _run(inputs, trace=False):
    import ml_dtypes
    from concourse.bass_utils import run_bass_kernel_spmd

    x = np.asarray(inputs["x"], np.float32)
    kv = np.asarray(inputs["kv"], np.float32)
    Wq = np.asarray(inputs["Wq"], np.float32)
    bq = np.asarray(inputs["bq"], np.float32)
    Wk = np.asarray(inputs["Wk"], np.float32)
    bk = np.asarray(inputs["bk"], np.float32)
    Wv = np.asarray(inputs["Wv"], np.float32)
    bv = np.asarray(inputs["bv"], np.float32)
    Wo = np.asarray(inputs["Wo"], np.float32)
    bo = np.asarray(inputs["bo"], np.float32)

    bf = ml_dtypes.bfloat16
    M0 = (np.arange(1024)[None, :] >= (np.arange(128)[:, None] + 512)
          ).astype(bf)

    # head-dim permutation: chunk c = [local head c | local head 4+c]
    # so each head's Q rows sit at the partition half of its KV group.
    hperm = np.concatenate(
        [np.concatenate([np.arange(c * 64, c * 64 + 64),
                         np.arange((4 + c) * 64, (4 + c) * 64 + 64)])
         for c in range(4)])  # [512] permutation of local head dims

    in_maps = []
    for core in range(NCORES):
        b, t = core // 4, core % 4
        bv_sh = bv[t * 128:(t + 1) * 128]
        bvt = np.broadcast_to(bv_sh[None, :], (128, 128)).astype(np.float32)
        wq_sh = Wq[:, t * 512:(t + 1) * 512][:, hperm]
        wo_sh = Wo[t * 512:(t + 1) * 512, :][hperm, :]
        bq_sh = bq[t * 512:(t + 1) * 512][hperm]
        in_maps.append({
            "xT": np.ascontiguousarray(x[b].T).astype(bf),
            "kvT": np.ascontiguousarray(kv[b].T).astype(bf),
            "wq": wq_sh.astype(bf),
            "wk": Wk[:, t * 128:(t + 1) * 128].astype(bf),
            "wv": Wv[:, t * 128:(t + 1) * 128].astype(bf),
            "wo": np.ascontiguousarray(wo_sh).astype(bf),
            "bq": np.ascontiguousarray(bq_sh.reshape(4, 128).T),
            "bk": bk[t * 128:(t + 1) * 128].reshape(128, 1).copy(),
            "bvt": np.ascontiguousarray(bvt),
            "m0": M0,
        })

    nc = _get_nc()
    res = run_bass_kernel_spmd(nc, in_maps, core_ids=list(range(NCORES)),
                               trace=trace)
    parts = [np.asarray(res.results[i]["out"], np.float32)
             for i in range(NCORES)]
    out = np.stack([parts[0] + parts[1] + parts[2] + parts[3],
                    parts[4] + parts[5] + parts[6] + parts[7]])
    out += bo[None, None, :]
    return out.astype(np.float32), res
